# revision 26
# baseline (speedup 1.0000x reference)
"""BlockedEllLinear TRN2 kernel (8 NeuronCores, token-parallel).

out = x @ (W * (1 + expand(block_mask))).T + bias
    = x @ Weff.T + bias      (the sparse and dense paths fuse: Weff = W*(1+M))

Sharding: pure data-parallel over tokens (8 groups of 1024). All heavy
layout work happens on the host so the device runs a bare bf16 matmul
at the PE roofline:
  - host: Weff = W*(1+M) in f32, cast bf16, laid out tile-order
    [op, p, kb, o] (one contiguous 1MB panel per 128 out-features);
    x cast bf16 and laid out [p, kb, t] per core (xT resident in SBUF);
    bias laid out [p, op] so it is a per-partition scalar on the device.
  - device per core: out.T[o, t] = sum_kb WeffT[kb,o-panel].T @ xT[kb, t]
    accumulated in PSUM over the full contraction (32 K-blocks), 2 banks
    of N=512 per o-panel. The first 4 o-panels advance together across
    all 8 PSUM banks ("fill phase") so each arriving xT chunk enables 4
    panels' worth of matmuls while the 8.4MB xT lands; the remaining 28
    panels run serially at the MM roofline. Bias is added during the
    PSUM->SBUF evacuation (DVE/ACT alternating, per-partition scalar —
    zero TensorE overhead). xT streams per-K-block on the two HWDGE
    rings (sync+scalar), weight panels on the gpsimd SWDGE queues,
    stores on the ACT ring; the last panel evacuates in quarter chunks
    on the idle sync ring to shorten the tail.
  - host: gather = per-core transpose + concat (out.T -> out).

PE work per core: 32 o-panels x 32 K-blocks x 2 = 2048 matmuls
[K=128]x[M=128]x[N=512] bf16 @ 216ns => ~444us busy, ~472us measured
(~6.5us NEFF preamble + DMA-bound fill + ~6us tail).
"""

import numpy as np
from ml_dtypes import bfloat16

import concourse.bass as bass
import concourse.mybir as mybir
import concourse.tile as tile
from concourse import bacc, bass_utils

F32 = mybir.dt.float32
BF16 = mybir.dt.bfloat16

TOKENS, IN_F, OUT_F = 8192, 4096, 4096
BLK = 16
N_CORES = 8
T_c = TOKENS // N_CORES  # 1024 tokens per core
KB = IN_F // 128  # 32 contraction blocks
OP = OUT_F // 128  # 32 out-feature panels
NH = T_c // 512  # 2 PSUM banks per o-panel


def _emit(tc, xt_c, w_c, bias_c, out_c):
    nc = tc.nc

    from contextlib import ExitStack

    ctx = ExitStack()
    with ctx:
        const_pool = ctx.enter_context(tc.tile_pool(name="const", bufs=1))
        x_pool = ctx.enter_context(tc.tile_pool(name="xres", bufs=1))
        w_pool = ctx.enter_context(tc.tile_pool(name="wst", bufs=5))
        psum_pool = ctx.enter_context(tc.tile_pool(name="ps", bufs=8, space="PSUM"))
        out_pool = ctx.enter_context(tc.tile_pool(name="ob", bufs=8))

        FILLG = 4  # panels interleaved during the fill phase

        bias_sb = const_pool.tile([128, OP], F32)
        nc.scalar.dma_start(bias_sb, bias_c)

        # resident xT: [p, kb, t]; fine-grained per-K-block DMAs (256KB, 2KB
        # lines) alternating across both HWDGE rings (sync + scalar) so the
        # x stream gets 2 of the 3 active rings' share of the SDMA engines
        xt = x_pool.tile([128, KB, T_c], BF16)
        for kb in range(KB):
            eng = nc.sync if kb % 2 == 0 else nc.scalar
            eng.dma_start(xt[:, kb, :], xt_c[:, kb, :])

        # fill panels' weights via SWDGE in two halves each: the low-kb
        # halves (all the fill needs for its first ~7us) land first, the
        # high-kb halves queue behind them — halving the early w traffic
        # that competes with the xt stream
        wts = []
        for p in range(FILLG):
            wt = w_pool.tile([128, KB, 128], BF16, tag="w", name=f"w{p}")
            if p == 0:
                # first piece quarter-sized so the first matmul fires sooner
                nc.gpsimd.dma_start(wt[:, 0 : KB // 4, :], w_c[p][:, 0 : KB // 4, :])
                nc.gpsimd.dma_start(
                    wt[:, KB // 4 : KB // 2, :], w_c[p][:, KB // 4 : KB // 2, :]
                )
            else:
                nc.gpsimd.dma_start(wt[:, 0 : KB // 2, :], w_c[p][:, 0 : KB // 2, :])
            wts.append(wt)
        for p in range(FILLG):
            nc.gpsimd.dma_start(
                wts[p][:, KB // 2 : KB, :], w_c[p][:, KB // 2 : KB, :]
            )

        # fill phase: first 4 panels advance together (all 8 PSUM banks) so
        # each arriving xt chunk enables 4 panels' worth of matmuls — the PE
        # absorbs most of the x-landing window instead of idling
        fill_pss = [
            [
                psum_pool.tile([128, 512], F32, tag="ps", name=f"ps{p}_{h}")
                for h in range(NH)
            ]
            for p in range(FILLG)
        ]
        for kb in range(KB):
            for p in range(FILLG):
                for h in range(NH):
                    nc.tensor.matmul(
                        fill_pss[p][h],
                        wts[p][:, kb, :],
                        xt[:, kb, h * 512 : (h + 1) * 512],
                        start=(kb == 0),
                        stop=(kb == KB - 1),
                    )
        for p in range(FILLG):
            for h in range(NH):
                ob = out_pool.tile([128, 512], BF16, tag="ob", name=f"ob{p}_{h}")
                if (p * NH + h) % 2 == 0:
                    nc.vector.tensor_scalar_add(ob, fill_pss[p][h], bias_sb[:, p : p + 1])
                else:
                    nc.scalar.add(ob, fill_pss[p][h], bias_sb[:, p : p + 1])
                nc.scalar.dma_start(
                    out_c[p * 128 : (p + 1) * 128, h * 512 : (h + 1) * 512], ob
                )

        for op in range(FILLG, OP):
            wt = w_pool.tile([128, KB, 128], BF16, tag="w", name=f"w{op}")
            nc.gpsimd.dma_start(wt, w_c[op])
            pss = [
                psum_pool.tile([128, 512], F32, tag="ps", name=f"ps{op}_{h}")
                for h in range(NH)
            ]
            for kb in range(KB):
                for h in range(NH):
                    nc.tensor.matmul(
                        pss[h],
                        wt[:, kb, :],
                        xt[:, kb, h * 512 : (h + 1) * 512],
                        start=(kb == 0),
                        stop=(kb == KB - 1),
                    )
            for h in range(NH):
                if op == OP - 1:
                    # last panel: chunked evacuation + stores on the idle
                    # sync ring to shorten the kernel tail
                    for q in range(2):
                        ob = out_pool.tile(
                            [128, 256], BF16, tag="obq", name=f"ob{op}_{h}_{q}"
                        )
                        if q % 2 == 0:
                            nc.vector.tensor_scalar_add(
                                ob, pss[h][:, q * 256 : (q + 1) * 256],
                                bias_sb[:, op : op + 1],
                            )
                        else:
                            nc.scalar.add(
                                ob, pss[h][:, q * 256 : (q + 1) * 256],
                                bias_sb[:, op : op + 1],
                            )
                        nc.sync.dma_start(
                            out_c[
                                op * 128 : (op + 1) * 128,
                                h * 512 + q * 256 : h * 512 + (q + 1) * 256,
                            ],
                            ob,
                        )
                    continue
                ob = out_pool.tile([128, 512], BF16, tag="ob", name=f"ob{op}_{h}")
                # bias-add fused into the PSUM evacuation; alternate DVE/ACT
                if (op * NH + h) % 2 == 0:
                    nc.vector.tensor_scalar_add(ob, pss[h], bias_sb[:, op : op + 1])
                else:
                    nc.scalar.add(ob, pss[h], bias_sb[:, op : op + 1])
                nc.scalar.dma_start(
                    out_c[op * 128 : (op + 1) * 128, h * 512 : (h + 1) * 512], ob
                )


_NC_CACHE = {}


def _get_nc():
    if "nc" not in _NC_CACHE:
        nc = bacc.Bacc(
            "TRN2",
            target_bir_lowering=False,
            debug=False,
            enable_asserts=False,
            num_devices=N_CORES,
        )
        xt_c = nc.dram_tensor("xt_c", [128, KB, T_c], BF16, kind="ExternalInput").ap()
        w_c = nc.dram_tensor("w_c", [OP, 128, KB, 128], BF16, kind="ExternalInput").ap()
        bias_c = nc.dram_tensor("bias_c", [128, OP], F32, kind="ExternalInput").ap()
        out_c = nc.dram_tensor("out_c", [OUT_F, T_c], BF16, kind="ExternalOutput").ap()
        with tile.TileContext(nc) as tc:
            _emit(tc, xt_c, w_c, bias_c, out_c)
        nc.compile()
        _NC_CACHE["nc"] = nc
    return _NC_CACHE["nc"]


def _make_in_maps(x, weight, bias, block_mask):
    x = np.ascontiguousarray(x, dtype=np.float32)
    weight = np.ascontiguousarray(weight, dtype=np.float32)
    bias = np.ascontiguousarray(bias, dtype=np.float32)
    maskf = 1.0 + np.asarray(block_mask).astype(np.float32)

    # Weff[o, i] = W[o, i] * (1 + M)[o//16, i//16], bf16, tile-order
    # wh[op, p, kb, o] = Weff[op*128+o, kb*128+p]
    weff = (weight.reshape(OUT_F // BLK, BLK, IN_F // BLK, BLK) * maskf[:, None, :, None]).reshape(
        OUT_F, IN_F
    )
    wh = np.ascontiguousarray(
        weff.astype(bfloat16).reshape(OP, 128, KB, 128).transpose(0, 3, 2, 1)
    )

    # xh[c][p, kb, t] = x[c*T_c + t, kb*128 + p]
    xb = x.astype(bfloat16)
    # bias_h[p, op] = bias[op*128 + p]
    bias_h = np.ascontiguousarray(bias.reshape(OP, 128).T)

    in_maps = []
    for cid in range(N_CORES):
        xc = xb[cid * T_c : (cid + 1) * T_c].reshape(T_c, KB, 128)
        in_maps.append(
            {
                "xt_c": np.ascontiguousarray(xc.transpose(2, 1, 0)),
                "w_c": wh,
                "bias_c": bias_h,
            }
        )
    return in_maps


def _gather(results):
    out = np.empty((TOKENS, OUT_F), np.float32)
    for cid in range(N_CORES):
        out[cid * T_c : (cid + 1) * T_c, :] = results[cid]["out_c"].T.astype(np.float32)
    return out


def kernel(x, weight, bias, block_mask):
    nc = _get_nc()
    in_maps = _make_in_maps(x, weight, bias, block_mask)
    res = bass_utils.run_bass_kernel_spmd(
        nc, in_maps, core_ids=list(range(N_CORES)), trace=False
    )
    return _gather(res.results)


# revision 27
# speedup vs baseline: 1.0026x; 1.0026x over previous
"""BlockedEllLinear TRN2 kernel (8 NeuronCores, token-parallel).

out = x @ (W * (1 + expand(block_mask))).T + bias
    = x @ Weff.T + bias      (the sparse and dense paths fuse: Weff = W*(1+M))

Sharding: pure data-parallel over tokens (8 groups of 1024). All heavy
layout work happens on the host so the device runs a bare bf16 matmul
at the PE roofline:
  - host: Weff = W*(1+M) in f32, cast bf16, laid out tile-order
    [op, p, kb, o] (one contiguous 1MB panel per 128 out-features);
    x cast bf16 and laid out [p, kb, t] per core (xT resident in SBUF);
    bias laid out [p, op] so it is a per-partition scalar on the device.
  - device per core: out.T[o, t] = sum_kb WeffT[kb,o-panel].T @ xT[kb, t]
    accumulated in PSUM over the full contraction (32 K-blocks), 2 banks
    of N=512 per o-panel. The first 4 o-panels advance together across
    all 8 PSUM banks ("fill phase") so each arriving xT chunk enables 4
    panels' worth of matmuls while the 8.4MB xT lands; the remaining 28
    panels run serially at the MM roofline. Bias is added during the
    PSUM->SBUF evacuation (DVE/ACT alternating, per-partition scalar —
    zero TensorE overhead). xT streams per-K-block on the two HWDGE
    rings (sync+scalar), weight panels on the gpsimd SWDGE queues,
    stores on the ACT ring; the last panel evacuates in quarter chunks
    on the idle sync ring to shorten the tail.
  - host: gather = per-core transpose + concat (out.T -> out).

PE work per core: 32 o-panels x 32 K-blocks x 2 = 2048 matmuls
[K=128]x[M=128]x[N=512] bf16 @ 216ns => ~444us busy, ~472us measured
(~6.5us NEFF preamble + DMA-bound fill + ~6us tail).
"""

import numpy as np
from ml_dtypes import bfloat16

import concourse.bass as bass
import concourse.mybir as mybir
import concourse.tile as tile
from concourse import bacc, bass_utils

F32 = mybir.dt.float32
BF16 = mybir.dt.bfloat16

TOKENS, IN_F, OUT_F = 8192, 4096, 4096
BLK = 16
N_CORES = 8
T_c = TOKENS // N_CORES  # 1024 tokens per core
KB = IN_F // 128  # 32 contraction blocks
OP = OUT_F // 128  # 32 out-feature panels
NH = T_c // 512  # 2 PSUM banks per o-panel


def _emit(tc, xt_c, w_c, bias_c, out_c):
    nc = tc.nc

    from contextlib import ExitStack

    ctx = ExitStack()
    with ctx:
        const_pool = ctx.enter_context(tc.tile_pool(name="const", bufs=1))
        x_pool = ctx.enter_context(tc.tile_pool(name="xres", bufs=1))
        w_pool = ctx.enter_context(tc.tile_pool(name="wst", bufs=5))
        psum_pool = ctx.enter_context(tc.tile_pool(name="ps", bufs=8, space="PSUM"))
        out_pool = ctx.enter_context(tc.tile_pool(name="ob", bufs=8))

        FILLG = 4  # panels interleaved during the fill phase

        bias_sb = const_pool.tile([128, OP], F32)
        nc.scalar.dma_start(bias_sb, bias_c)

        # resident xT: [p, kb, t]; fine-grained per-K-block DMAs (256KB, 2KB
        # lines) alternating across both HWDGE rings (sync + scalar) so the
        # x stream gets 2 of the 3 active rings' share of the SDMA engines
        xt = x_pool.tile([128, KB, T_c], BF16)
        for kb in range(KB):
            eng = nc.sync if kb % 2 == 0 else nc.scalar
            eng.dma_start(xt[:, kb, :], xt_c[:, kb, :])

        # fill panels' weights via SWDGE in two halves each: the low-kb
        # halves (all the fill needs for its first ~7us) land first, the
        # high-kb halves queue behind them — halving the early w traffic
        # that competes with the xt stream
        wts = []
        for p in range(FILLG):
            wt = w_pool.tile([128, KB, 128], BF16, tag="w", name=f"w{p}")
            nc.gpsimd.dma_start(wt[:, 0 : KB // 2, :], w_c[p][:, 0 : KB // 2, :])
            wts.append(wt)
        for p in range(FILLG):
            nc.gpsimd.dma_start(
                wts[p][:, KB // 2 : KB, :], w_c[p][:, KB // 2 : KB, :]
            )

        # fill phase: first 4 panels advance together (all 8 PSUM banks) so
        # each arriving xt chunk enables 4 panels' worth of matmuls — the PE
        # absorbs most of the x-landing window instead of idling
        fill_pss = [
            [
                psum_pool.tile([128, 512], F32, tag="ps", name=f"ps{p}_{h}")
                for h in range(NH)
            ]
            for p in range(FILLG)
        ]
        for kb in range(KB):
            for p in range(FILLG):
                for h in range(NH):
                    nc.tensor.matmul(
                        fill_pss[p][h],
                        wts[p][:, kb, :],
                        xt[:, kb, h * 512 : (h + 1) * 512],
                        start=(kb == 0),
                        stop=(kb == KB - 1),
                    )
        for p in range(FILLG):
            for h in range(NH):
                ob = out_pool.tile([128, 512], F32, tag="ob", name=f"ob{p}_{h}")
                if (p * NH + h) % 2 == 0:
                    nc.vector.tensor_scalar_add(ob, fill_pss[p][h], bias_sb[:, p : p + 1])
                else:
                    nc.scalar.add(ob, fill_pss[p][h], bias_sb[:, p : p + 1])
                nc.scalar.dma_start(
                    out_c[p * 128 : (p + 1) * 128, h * 512 : (h + 1) * 512], ob
                )

        for op in range(FILLG, OP):
            wt = w_pool.tile([128, KB, 128], BF16, tag="w", name=f"w{op}")
            nc.gpsimd.dma_start(wt, w_c[op])
            pss = [
                psum_pool.tile([128, 512], F32, tag="ps", name=f"ps{op}_{h}")
                for h in range(NH)
            ]
            for kb in range(KB):
                for h in range(NH):
                    nc.tensor.matmul(
                        pss[h],
                        wt[:, kb, :],
                        xt[:, kb, h * 512 : (h + 1) * 512],
                        start=(kb == 0),
                        stop=(kb == KB - 1),
                    )
            for h in range(NH):
                if op == OP - 1:
                    # last panel: chunked evacuation + stores on the idle
                    # sync ring to shorten the kernel tail
                    for q in range(2):
                        ob = out_pool.tile(
                            [128, 256], F32, tag="obq", name=f"ob{op}_{h}_{q}"
                        )
                        if q % 2 == 0:
                            nc.vector.tensor_scalar_add(
                                ob, pss[h][:, q * 256 : (q + 1) * 256],
                                bias_sb[:, op : op + 1],
                            )
                        else:
                            nc.scalar.add(
                                ob, pss[h][:, q * 256 : (q + 1) * 256],
                                bias_sb[:, op : op + 1],
                            )
                        nc.sync.dma_start(
                            out_c[
                                op * 128 : (op + 1) * 128,
                                h * 512 + q * 256 : h * 512 + (q + 1) * 256,
                            ],
                            ob,
                        )
                    continue
                ob = out_pool.tile([128, 512], F32, tag="ob", name=f"ob{op}_{h}")
                # bias-add fused into the PSUM evacuation; alternate DVE/ACT
                if (op * NH + h) % 2 == 0:
                    nc.vector.tensor_scalar_add(ob, pss[h], bias_sb[:, op : op + 1])
                else:
                    nc.scalar.add(ob, pss[h], bias_sb[:, op : op + 1])
                nc.scalar.dma_start(
                    out_c[op * 128 : (op + 1) * 128, h * 512 : (h + 1) * 512], ob
                )


_NC_CACHE = {}


def _get_nc():
    if "nc" not in _NC_CACHE:
        nc = bacc.Bacc(
            "TRN2",
            target_bir_lowering=False,
            debug=False,
            enable_asserts=False,
            num_devices=N_CORES,
        )
        xt_c = nc.dram_tensor("xt_c", [128, KB, T_c], BF16, kind="ExternalInput").ap()
        w_c = nc.dram_tensor("w_c", [OP, 128, KB, 128], BF16, kind="ExternalInput").ap()
        bias_c = nc.dram_tensor("bias_c", [128, OP], F32, kind="ExternalInput").ap()
        out_c = nc.dram_tensor("out_c", [OUT_F, T_c], F32, kind="ExternalOutput").ap()
        with tile.TileContext(nc) as tc:
            _emit(tc, xt_c, w_c, bias_c, out_c)
        nc.compile()
        _NC_CACHE["nc"] = nc
    return _NC_CACHE["nc"]


def _make_in_maps(x, weight, bias, block_mask):
    x = np.ascontiguousarray(x, dtype=np.float32)
    weight = np.ascontiguousarray(weight, dtype=np.float32)
    bias = np.ascontiguousarray(bias, dtype=np.float32)
    maskf = 1.0 + np.asarray(block_mask).astype(np.float32)

    # Weff[o, i] = W[o, i] * (1 + M)[o//16, i//16], bf16, tile-order
    # wh[op, p, kb, o] = Weff[op*128+o, kb*128+p]
    weff = (weight.reshape(OUT_F // BLK, BLK, IN_F // BLK, BLK) * maskf[:, None, :, None]).reshape(
        OUT_F, IN_F
    )
    wh = np.ascontiguousarray(
        weff.astype(bfloat16).reshape(OP, 128, KB, 128).transpose(0, 3, 2, 1)
    )

    # xh[c][p, kb, t] = x[c*T_c + t, kb*128 + p]
    xb = x.astype(bfloat16)
    # bias_h[p, op] = bias[op*128 + p]
    bias_h = np.ascontiguousarray(bias.reshape(OP, 128).T)

    in_maps = []
    for cid in range(N_CORES):
        xc = xb[cid * T_c : (cid + 1) * T_c].reshape(T_c, KB, 128)
        in_maps.append(
            {
                "xt_c": np.ascontiguousarray(xc.transpose(2, 1, 0)),
                "w_c": wh,
                "bias_c": bias_h,
            }
        )
    return in_maps


def _gather(results):
    out = np.empty((TOKENS, OUT_F), np.float32)
    for cid in range(N_CORES):
        out[cid * T_c : (cid + 1) * T_c, :] = results[cid]["out_c"].T
    return out


def kernel(x, weight, bias, block_mask):
    nc = _get_nc()
    in_maps = _make_in_maps(x, weight, bias, block_mask)
    res = bass_utils.run_bass_kernel_spmd(
        nc, in_maps, core_ids=list(range(N_CORES)), trace=False
    )
    return _gather(res.results)


# revision 29
# speedup vs baseline: 1.0055x; 1.0029x over previous
"""BlockedEllLinear TRN2 kernel (8 NeuronCores, token-parallel).

out = x @ (W * (1 + expand(block_mask))).T + bias
    = x @ Weff.T + bias      (the sparse and dense paths fuse: Weff = W*(1+M))

Sharding: pure data-parallel over tokens (8 groups of 1024). All heavy
layout work happens on the host so the device runs a bare bf16 matmul
at the PE roofline:
  - host: Weff = W*(1+M) in f32, cast bf16, laid out tile-order
    [op, p, kb, o] (one contiguous 1MB panel per 128 out-features);
    x cast bf16 and laid out [p, kb, t] per core (xT resident in SBUF);
    bias laid out [p, op] so it is a per-partition scalar on the device.
  - device per core: out.T[o, t] = sum_kb WeffT[kb,o-panel].T @ xT[kb, t]
    accumulated in PSUM over the full contraction (32 K-blocks), 2 banks
    of N=512 per o-panel. The first 4 o-panels advance together across
    all 8 PSUM banks ("fill phase") so each arriving xT chunk enables 4
    panels' worth of matmuls while the 8.4MB xT lands; the remaining 28
    panels run serially at the MM roofline. Bias is added during the
    PSUM->SBUF evacuation (DVE/ACT alternating, per-partition scalar —
    zero TensorE overhead). xT streams per-K-block on the two HWDGE
    rings (sync+scalar), weight panels on the gpsimd SWDGE queues,
    stores on the ACT ring; the last panel evacuates in quarter chunks
    on the idle sync ring to shorten the tail.
  - host: gather = per-core transpose + concat (out.T -> out).

The fill panels' weights stream via SWDGE in two halves each, low-kb
halves first, so early HBM bandwidth goes to the xT stream.

PE work per core: 32 o-panels x 32 K-blocks x 2 = 2048 matmuls
[K=128]x[M=128]x[N=512] bf16 @ 216ns => ~443us busy, ~470us measured
(~6.5us NEFF preamble + DMA-bound fill + ~7us tail).
"""

import numpy as np
from ml_dtypes import bfloat16

import concourse.bass as bass
import concourse.mybir as mybir
import concourse.tile as tile
from concourse import bacc, bass_utils

F32 = mybir.dt.float32
BF16 = mybir.dt.bfloat16

TOKENS, IN_F, OUT_F = 8192, 4096, 4096
BLK = 16
N_CORES = 8
T_c = TOKENS // N_CORES  # 1024 tokens per core
KB = IN_F // 128  # 32 contraction blocks
OP = OUT_F // 128  # 32 out-feature panels
NH = T_c // 512  # 2 PSUM banks per o-panel


def _emit(tc, xt_c, w_c, bias_c, out_c):
    nc = tc.nc

    from contextlib import ExitStack

    ctx = ExitStack()
    with ctx:
        const_pool = ctx.enter_context(tc.tile_pool(name="const", bufs=1))
        x_pool = ctx.enter_context(tc.tile_pool(name="xres", bufs=1))
        w_pool = ctx.enter_context(tc.tile_pool(name="wst", bufs=5))
        psum_pool = ctx.enter_context(tc.tile_pool(name="ps", bufs=8, space="PSUM"))
        out_pool = ctx.enter_context(tc.tile_pool(name="ob", bufs=8))

        FILLG = 4  # panels interleaved during the fill phase

        bias_sb = const_pool.tile([128, OP], F32)
        nc.scalar.dma_start(bias_sb, bias_c)

        # resident xT: [p, kb, t]; fine-grained per-K-block DMAs (256KB, 2KB
        # lines) alternating across both HWDGE rings (sync + scalar) so the
        # x stream gets 2 of the 3 active rings' share of the SDMA engines
        xt = x_pool.tile([128, KB, T_c], BF16)
        for kb in range(KB):
            eng = nc.sync if kb % 2 == 0 else nc.scalar
            eng.dma_start(xt[:, kb, :], xt_c[:, kb, :])

        # fill panels' weights via SWDGE in two halves each: the low-kb
        # halves (all the fill needs for its first ~7us) land first, the
        # high-kb halves queue behind them — halving the early w traffic
        # that competes with the xt stream
        wts = []
        for p in range(FILLG):
            wt = w_pool.tile([128, KB, 128], BF16, tag="w", name=f"w{p}")
            wts.append(wt)
        Q = KB // 4
        for q in range(4):
            for p in range(FILLG):
                nc.gpsimd.dma_start(
                    wts[p][:, q * Q : (q + 1) * Q, :],
                    w_c[p][:, q * Q : (q + 1) * Q, :],
                )

        # fill phase: first 4 panels advance together (all 8 PSUM banks) so
        # each arriving xt chunk enables 4 panels' worth of matmuls — the PE
        # absorbs most of the x-landing window instead of idling
        fill_pss = [
            [
                psum_pool.tile([128, 512], F32, tag="ps", name=f"ps{p}_{h}")
                for h in range(NH)
            ]
            for p in range(FILLG)
        ]
        for kb in range(KB):
            for p in range(FILLG):
                for h in range(NH):
                    nc.tensor.matmul(
                        fill_pss[p][h],
                        wts[p][:, kb, :],
                        xt[:, kb, h * 512 : (h + 1) * 512],
                        start=(kb == 0),
                        stop=(kb == KB - 1),
                    )
        for p in range(FILLG):
            for h in range(NH):
                ob = out_pool.tile([128, 512], F32, tag="ob", name=f"ob{p}_{h}")
                if (p * NH + h) % 2 == 0:
                    nc.vector.tensor_scalar_add(ob, fill_pss[p][h], bias_sb[:, p : p + 1])
                else:
                    nc.scalar.add(ob, fill_pss[p][h], bias_sb[:, p : p + 1])
                nc.scalar.dma_start(
                    out_c[p * 128 : (p + 1) * 128, h * 512 : (h + 1) * 512], ob
                )

        for op in range(FILLG, OP):
            wt = w_pool.tile([128, KB, 128], BF16, tag="w", name=f"w{op}")
            nc.gpsimd.dma_start(wt, w_c[op])
            pss = [
                psum_pool.tile([128, 512], F32, tag="ps", name=f"ps{op}_{h}")
                for h in range(NH)
            ]
            for kb in range(KB):
                for h in range(NH):
                    nc.tensor.matmul(
                        pss[h],
                        wt[:, kb, :],
                        xt[:, kb, h * 512 : (h + 1) * 512],
                        start=(kb == 0),
                        stop=(kb == KB - 1),
                    )
            for h in range(NH):
                if op == OP - 1:
                    # last panel: chunked evacuation + stores on the idle
                    # sync ring to shorten the kernel tail
                    for q in range(2):
                        ob = out_pool.tile(
                            [128, 256], F32, tag="obq", name=f"ob{op}_{h}_{q}"
                        )
                        if q % 2 == 0:
                            nc.vector.tensor_scalar_add(
                                ob, pss[h][:, q * 256 : (q + 1) * 256],
                                bias_sb[:, op : op + 1],
                            )
                        else:
                            nc.scalar.add(
                                ob, pss[h][:, q * 256 : (q + 1) * 256],
                                bias_sb[:, op : op + 1],
                            )
                        nc.sync.dma_start(
                            out_c[
                                op * 128 : (op + 1) * 128,
                                h * 512 + q * 256 : h * 512 + (q + 1) * 256,
                            ],
                            ob,
                        )
                    continue
                ob = out_pool.tile([128, 512], F32, tag="ob", name=f"ob{op}_{h}")
                # bias-add fused into the PSUM evacuation; alternate DVE/ACT
                if (op * NH + h) % 2 == 0:
                    nc.vector.tensor_scalar_add(ob, pss[h], bias_sb[:, op : op + 1])
                else:
                    nc.scalar.add(ob, pss[h], bias_sb[:, op : op + 1])
                nc.scalar.dma_start(
                    out_c[op * 128 : (op + 1) * 128, h * 512 : (h + 1) * 512], ob
                )


_NC_CACHE = {}


def _get_nc():
    if "nc" not in _NC_CACHE:
        nc = bacc.Bacc(
            "TRN2",
            target_bir_lowering=False,
            debug=False,
            enable_asserts=False,
            num_devices=N_CORES,
        )
        xt_c = nc.dram_tensor("xt_c", [128, KB, T_c], BF16, kind="ExternalInput").ap()
        w_c = nc.dram_tensor("w_c", [OP, 128, KB, 128], BF16, kind="ExternalInput").ap()
        bias_c = nc.dram_tensor("bias_c", [128, OP], F32, kind="ExternalInput").ap()
        out_c = nc.dram_tensor("out_c", [OUT_F, T_c], F32, kind="ExternalOutput").ap()
        with tile.TileContext(nc) as tc:
            _emit(tc, xt_c, w_c, bias_c, out_c)
        nc.compile()
        _NC_CACHE["nc"] = nc
    return _NC_CACHE["nc"]


def _make_in_maps(x, weight, bias, block_mask):
    x = np.ascontiguousarray(x, dtype=np.float32)
    weight = np.ascontiguousarray(weight, dtype=np.float32)
    bias = np.ascontiguousarray(bias, dtype=np.float32)
    maskf = 1.0 + np.asarray(block_mask).astype(np.float32)

    # Weff[o, i] = W[o, i] * (1 + M)[o//16, i//16], bf16, tile-order
    # wh[op, p, kb, o] = Weff[op*128+o, kb*128+p]
    weff = (weight.reshape(OUT_F // BLK, BLK, IN_F // BLK, BLK) * maskf[:, None, :, None]).reshape(
        OUT_F, IN_F
    )
    wh = np.ascontiguousarray(
        weff.astype(bfloat16).reshape(OP, 128, KB, 128).transpose(0, 3, 2, 1)
    )

    # xh[c][p, kb, t] = x[c*T_c + t, kb*128 + p]
    xb = x.astype(bfloat16)
    # bias_h[p, op] = bias[op*128 + p]
    bias_h = np.ascontiguousarray(bias.reshape(OP, 128).T)

    in_maps = []
    for cid in range(N_CORES):
        xc = xb[cid * T_c : (cid + 1) * T_c].reshape(T_c, KB, 128)
        in_maps.append(
            {
                "xt_c": np.ascontiguousarray(xc.transpose(2, 1, 0)),
                "w_c": wh,
                "bias_c": bias_h,
            }
        )
    return in_maps


def _gather(results):
    out = np.empty((TOKENS, OUT_F), np.float32)
    for cid in range(N_CORES):
        out[cid * T_c : (cid + 1) * T_c, :] = results[cid]["out_c"].T
    return out


def kernel(x, weight, bias, block_mask):
    nc = _get_nc()
    in_maps = _make_in_maps(x, weight, bias, block_mask)
    res = bass_utils.run_bass_kernel_spmd(
        nc, in_maps, core_ids=list(range(N_CORES)), trace=False
    )
    return _gather(res.results)


# revision 30
# speedup vs baseline: 1.0101x; 1.0046x over previous
"""BlockedEllLinear TRN2 kernel (8 NeuronCores, token-parallel).

out = x @ (W * (1 + expand(block_mask))).T + bias
    = x @ Weff.T + bias      (the sparse and dense paths fuse: Weff = W*(1+M))

Sharding: pure data-parallel over tokens (8 groups of 1024). All heavy
layout work happens on the host so the device runs a bare bf16 matmul
at the PE roofline:
  - host: Weff = W*(1+M) in f32, cast bf16, laid out tile-order
    [op, p, kb, o] (one contiguous 1MB panel per 128 out-features);
    x cast bf16 and laid out [p, kb, t] per core (xT resident in SBUF);
    bias laid out [p, op] so it is a per-partition scalar on the device.
  - device per core: out.T[o, t] = sum_kb WeffT[kb,o-panel].T @ xT[kb, t]
    accumulated in PSUM over the full contraction (32 K-blocks), 2 banks
    of N=512 per o-panel. The first 4 o-panels advance together across
    all 8 PSUM banks ("fill phase") so each arriving xT chunk enables 4
    panels' worth of matmuls while the 8.4MB xT lands; the remaining 28
    panels run serially at the MM roofline. Bias is added during the
    PSUM->SBUF evacuation (DVE/ACT alternating, per-partition scalar —
    zero TensorE overhead). xT streams per-K-block on the two HWDGE
    rings (sync+scalar), weight panels on the gpsimd SWDGE queues,
    stores on the ACT ring; the last panel evacuates in quarter chunks
    on the idle sync ring to shorten the tail.
  - host: gather = per-core transpose + concat (out.T -> out).

The fill panels' weights stream via SWDGE in four quarters each, all
low-kb quarters first, so early HBM bandwidth goes to the xT stream
(the SWDGE descriptor-gen pacing defers the rest past the window).

PE work per core: 32 o-panels x 32 K-blocks x 2 = 2048 matmuls
[K=128]x[M=128]x[N=512] bf16 @ 216ns => ~443us busy, ~469us measured
(~6.5us NEFF preamble + DMA-bound fill + ~7us tail).
"""

import numpy as np
from ml_dtypes import bfloat16

import concourse.bass as bass
import concourse.mybir as mybir
import concourse.tile as tile
from concourse import bacc, bass_utils

F32 = mybir.dt.float32
BF16 = mybir.dt.bfloat16

TOKENS, IN_F, OUT_F = 8192, 4096, 4096
BLK = 16
N_CORES = 8
T_c = TOKENS // N_CORES  # 1024 tokens per core
KB = IN_F // 128  # 32 contraction blocks
OP = OUT_F // 128  # 32 out-feature panels
NH = T_c // 512  # 2 PSUM banks per o-panel


def _emit(tc, xt_c, w_c, bias_c, out_c):
    nc = tc.nc

    from contextlib import ExitStack

    ctx = ExitStack()
    with ctx:
        const_pool = ctx.enter_context(tc.tile_pool(name="const", bufs=1))
        x_pool = ctx.enter_context(tc.tile_pool(name="xres", bufs=1))
        w_pool = ctx.enter_context(tc.tile_pool(name="wst", bufs=5))
        psum_pool = ctx.enter_context(tc.tile_pool(name="ps", bufs=8, space="PSUM"))
        out_pool = ctx.enter_context(tc.tile_pool(name="ob", bufs=8))

        FILLG = 4  # panels interleaved during the fill phase

        bias_sb = const_pool.tile([128, OP], F32)
        nc.scalar.dma_start(bias_sb, bias_c)

        # resident xT: [p, kb, t]; fine-grained per-K-block DMAs (256KB, 2KB
        # lines) alternating across both HWDGE rings (sync + scalar) so the
        # x stream gets 2 of the 3 active rings' share of the SDMA engines
        xt = x_pool.tile([128, KB, T_c], BF16)
        for kb in range(KB):
            eng = nc.sync if kb % 2 == 0 else nc.scalar
            eng.dma_start(xt[:, kb, :], xt_c[:, kb, :])

        # fill panels' weights via SWDGE in two halves each: the low-kb
        # halves (all the fill needs for its first ~7us) land first, the
        # high-kb halves queue behind them — halving the early w traffic
        # that competes with the xt stream
        wts = []
        for p in range(FILLG):
            wt = w_pool.tile([128, KB, 128], BF16, tag="w", name=f"w{p}")
            wts.append(wt)
        Q = KB // 4
        for q in range(4):
            for p in range(FILLG):
                nc.gpsimd.dma_start(
                    wts[p][:, q * Q : (q + 1) * Q, :],
                    w_c[p][:, q * Q : (q + 1) * Q, :],
                )

        # fill phase: first 4 panels advance together (all 8 PSUM banks) so
        # each arriving xt chunk enables 4 panels' worth of matmuls — the PE
        # absorbs most of the x-landing window instead of idling
        fill_pss = [
            [
                psum_pool.tile([128, 512], F32, tag="ps", name=f"ps{p}_{h}")
                for h in range(NH)
            ]
            for p in range(FILLG)
        ]
        for kb in range(KB):
            for p in range(FILLG):
                for h in range(NH):
                    nc.tensor.matmul(
                        fill_pss[p][h],
                        wts[p][:, kb, :],
                        xt[:, kb, h * 512 : (h + 1) * 512],
                        start=(kb == 0),
                        stop=(kb == KB - 1),
                    )
        for p in range(FILLG):
            for h in range(NH):
                ob = out_pool.tile([128, 512], F32, tag="ob", name=f"ob{p}_{h}")
                if (p * NH + h) % 2 == 0:
                    nc.vector.tensor_scalar_add(ob, fill_pss[p][h], bias_sb[:, p : p + 1])
                else:
                    nc.scalar.add(ob, fill_pss[p][h], bias_sb[:, p : p + 1])
                nc.scalar.dma_start(
                    out_c[p * 128 : (p + 1) * 128, h * 512 : (h + 1) * 512], ob
                )

        for op in range(FILLG, OP):
            wt = w_pool.tile([128, KB, 128], BF16, tag="w", name=f"w{op}")
            nc.gpsimd.dma_start(wt, w_c[op])
            pss = [
                psum_pool.tile([128, 512], F32, tag="ps", name=f"ps{op}_{h}")
                for h in range(NH)
            ]
            for kb in range(KB):
                for h in range(NH):
                    nc.tensor.matmul(
                        pss[h],
                        wt[:, kb, :],
                        xt[:, kb, h * 512 : (h + 1) * 512],
                        start=(kb == 0),
                        stop=(kb == KB - 1),
                    )
            for h in range(NH):
                if op == OP - 1:
                    # last panel: chunked evacuation + stores on the idle
                    # sync ring to shorten the kernel tail
                    for q in range(2):
                        ob = out_pool.tile(
                            [128, 256], F32, tag="obq", name=f"ob{op}_{h}_{q}"
                        )
                        if q % 2 == 0:
                            nc.vector.tensor_scalar_add(
                                ob, pss[h][:, q * 256 : (q + 1) * 256],
                                bias_sb[:, op : op + 1],
                            )
                        else:
                            nc.scalar.add(
                                ob, pss[h][:, q * 256 : (q + 1) * 256],
                                bias_sb[:, op : op + 1],
                            )
                        nc.sync.dma_start(
                            out_c[
                                op * 128 : (op + 1) * 128,
                                h * 512 + q * 256 : h * 512 + (q + 1) * 256,
                            ],
                            ob,
                        )
                    continue
                ob = out_pool.tile([128, 512], F32, tag="ob", name=f"ob{op}_{h}")
                # bias-add fused into the PSUM evacuation; alternate DVE/ACT
                if (op * NH + h) % 2 == 0:
                    nc.vector.tensor_scalar_add(ob, pss[h], bias_sb[:, op : op + 1])
                else:
                    nc.scalar.add(ob, pss[h], bias_sb[:, op : op + 1])
                nc.scalar.dma_start(
                    out_c[op * 128 : (op + 1) * 128, h * 512 : (h + 1) * 512], ob
                )


_NC_CACHE = {}


def _get_nc():
    if "nc" not in _NC_CACHE:
        nc = bacc.Bacc(
            "TRN2",
            target_bir_lowering=False,
            debug=False,
            enable_asserts=False,
            num_devices=N_CORES,
        )
        xt_c = nc.dram_tensor("xt_c", [128, KB, T_c], BF16, kind="ExternalInput").ap()
        w_c = nc.dram_tensor("w_c", [OP, 128, KB, 128], BF16, kind="ExternalInput").ap()
        bias_c = nc.dram_tensor("bias_c", [128, OP], F32, kind="ExternalInput").ap()
        out_c = nc.dram_tensor("out_c", [OUT_F, T_c], F32, kind="ExternalOutput").ap()
        with tile.TileContext(nc) as tc:
            _emit(tc, xt_c, w_c, bias_c, out_c)
        nc.compile()
        _NC_CACHE["nc"] = nc
    return _NC_CACHE["nc"]


def _make_in_maps(x, weight, bias, block_mask):
    x = np.ascontiguousarray(x, dtype=np.float32)
    weight = np.ascontiguousarray(weight, dtype=np.float32)
    bias = np.ascontiguousarray(bias, dtype=np.float32)
    maskf = 1.0 + np.asarray(block_mask).astype(np.float32)

    # Weff[o, i] = W[o, i] * (1 + M)[o//16, i//16], bf16, tile-order
    # wh[op, p, kb, o] = Weff[op*128+o, kb*128+p]
    weff = (weight.reshape(OUT_F // BLK, BLK, IN_F // BLK, BLK) * maskf[:, None, :, None]).reshape(
        OUT_F, IN_F
    )
    wh = np.ascontiguousarray(
        weff.astype(bfloat16).reshape(OP, 128, KB, 128).transpose(0, 3, 2, 1)
    )

    # xh[c][p, kb, t] = x[c*T_c + t, kb*128 + p]
    xb = x.astype(bfloat16)
    # bias_h[p, op] = bias[op*128 + p]
    bias_h = np.ascontiguousarray(bias.reshape(OP, 128).T)

    in_maps = []
    for cid in range(N_CORES):
        xc = xb[cid * T_c : (cid + 1) * T_c].reshape(T_c, KB, 128)
        in_maps.append(
            {
                "xt_c": np.ascontiguousarray(xc.transpose(2, 1, 0)),
                "w_c": wh,
                "bias_c": bias_h,
            }
        )
    return in_maps


def _gather(results):
    out = np.empty((TOKENS, OUT_F), np.float32)
    for cid in range(N_CORES):
        out[cid * T_c : (cid + 1) * T_c, :] = results[cid]["out_c"].T
    return out


def kernel(x, weight, bias, block_mask):
    nc = _get_nc()
    in_maps = _make_in_maps(x, weight, bias, block_mask)
    res = bass_utils.run_bass_kernel_spmd(
        nc, in_maps, core_ids=list(range(N_CORES)), trace=False
    )
    return _gather(res.results)
